# revision 1
# baseline (speedup 1.0000x reference)
"""Trainium2 Bass kernel for linear attention (elu+1 feature map).

Reference computation (B=4, N=M=8192, C=512, H=8, D=64):
    kv   = ref @ kv_w.T              -> k, v  [B,H,N,D]
    q    = tgt @ q_w.T               -> [B,H,M,D];  q,k -> elu(x)+1
    ctx  = sum_n k v^T per head      -> [B,H,D,D];  ksum = sum_n k
    x    = (q @ ctx) * SCALE / (1e-6 + q . ksum)
    out  = x @ proj_w.T + proj_b     -> [B,M,C]

Sharding: 8 cores = 4 batches x 2 row-halves. Each core computes partial
ctx/ksum from its half of N, pair-AllReduces the tiny per-head state, then
produces its half of M rows of the output.

Device dataflow (per core, R=4096 rows):
  phase 1:  kv row-major (lhsT = host-pretransposed refT tiles); elu+1 on k
            split as one DVE + two chained ACT ops; per-head-pair ctx matmul
            with a constant ones-column on v folding ksum into column 128.
  cc:       264KB pair AllReduce of the [128, 516] state.
  phase 2a: qT weights-stationary + elu (emitted before anything that waits
            on the collective so every engine stream stays busy during it).
  phase 2b: software-pipelined chunks: denom = Ksel.T @ qTe, reciprocal,
            PE-broadcast of recip to head partitions, x via block-diagonal
            ctx matmul, division fused into the PSUM->SBUF copyback, then
            out-proj (lhsT = xT) + host-broadcast bias.
"""

import numpy as np
import ml_dtypes

import concourse.bass as bass
import concourse.mybir as mybir
from concourse import bacc
from concourse.tile import TileContext
from concourse.bass import ts
from concourse.bass_utils import run_bass_kernel_spmd

B, N, M, C, H = 4, 8192, 8192, 512, 8
D = C // H
SCALE = D**-0.5
NCORES = 8
BF = mybir.dt.bfloat16
F32 = mybir.dt.float32

_CACHE = {}


def build(R_ref, R_q, num_devices, replica_groups, lookahead=2):
    """Emit the SPMD graph. R_ref/R_q = rows of the ref/target shard."""
    P = 128
    KC = C // P          # 4 c-chunks
    NT1 = R_ref // P     # phase-1 row tiles
    CH = 512             # phase-2 chunk (columns of rows)
    NCH = R_q // CH      # phase-2 chunks
    RT = CH // P         # row tiles per chunk
    NPAIR = H // 2       # head pairs
    CP = C + NPAIR       # 516: 4 pairs x 129 cols (128 ctx + 1 ksum)
    STATE = P * CP       # collective payload floats

    nc = bacc.Bacc("TRN2", target_bir_lowering=False, debug=False,
                   num_devices=num_devices)

    refT = nc.dram_tensor("refT", [C, R_ref], BF, kind="ExternalInput")
    tgtT = nc.dram_tensor("tgtT", [C, R_q], BF, kind="ExternalInput")
    kv_wT = nc.dram_tensor("kv_wT", [C, 2 * C], BF, kind="ExternalInput")
    q_wT = nc.dram_tensor("q_wT", [C, C], BF, kind="ExternalInput")
    proj_wT = nc.dram_tensor("proj_wT", [C, C], BF, kind="ExternalInput")
    bias_b = nc.dram_tensor("bias_b", [P, C], F32, kind="ExternalInput")
    E_const = nc.dram_tensor("E_const", [NPAIR, H, P], BF, kind="ExternalInput")
    out_ext = nc.dram_tensor("out", [R_q, C], F32, kind="ExternalOutput")
    cc_in = nc.dram_tensor("cc_in", [STATE], F32)
    cc_out = nc.dram_tensor("cc_out", [STATE], F32)

    with TileContext(nc) as tc:
        with (
            tc.tile_pool(name="res", bufs=1) as res,
            tc.tile_pool(name="mm", bufs=3, space="PSUM") as pmm,
            tc.tile_pool(name="kv", bufs=4) as kvp,
            tc.tile_pool(name="tmp", bufs=4) as tmp,
            tc.tile_pool(name="rc", bufs=3) as rcp,
            tc.tile_pool(name="qte", bufs=1) as qtep,
            tc.tile_pool(name="xt", bufs=1 + lookahead) as xtp,
            tc.tile_pool(name="o", bufs=6) as op_,
        ):
            # ---- resident inputs ----
            # kv weights (k-half first) + refT pieces first so phase 1 can
            # start after ~1MB of DMA instead of the full 18MB.
            NPIECE = 8
            PC_R = R_ref // NPIECE
            PC_Q = R_q // NPIECE
            kvw_sb = []
            for kc in range(KC):
                t = res.tile([P, 2 * C], BF, tag=f"kvw{kc}")
                nc.sync.dma_start(t[:, 0:C], kv_wT[ts(kc, P), 0:C])
                kvw_sb.append(t)
            refT_sb = [res.tile([P, R_ref], BF, tag=f"refT{kc}",
                                name=f"refT_sb{kc}") for kc in range(KC)]
            for kc in range(KC):
                nc.sync.dma_start(refT_sb[kc][:, ts(0, PC_R)],
                                  refT[ts(kc, P), ts(0, PC_R)])
            for kc in range(KC):
                nc.sync.dma_start(kvw_sb[kc][:, C : 2 * C],
                                  kv_wT[ts(kc, P), C : 2 * C])
            for pc in range(1, NPIECE):
                for kc in range(KC):
                    nc.sync.dma_start(refT_sb[kc][:, ts(pc, PC_R)],
                                      refT[ts(kc, P), ts(pc, PC_R)])
            qw_sb = []
            pw_sb = []
            for kc in range(KC):
                t = res.tile([P, C], BF, tag=f"qw{kc}")
                nc.sync.dma_start(t[:], q_wT[ts(kc, P), :])
                qw_sb.append(t)
                t = res.tile([P, C], BF, tag=f"pw{kc}")
                nc.sync.dma_start(t[:], proj_wT[ts(kc, P), :])
                pw_sb.append(t)
            tgtT_sb = [res.tile([P, R_q], BF, tag=f"tgtT{kc}",
                                name=f"tgtT_sb{kc}") for kc in range(KC)]
            for pc in range(NPIECE):
                for kc in range(KC):
                    nc.sync.dma_start(tgtT_sb[kc][:, ts(pc, PC_Q)],
                                      tgtT[ts(kc, P), ts(pc, PC_Q)])
            bias_sb = res.tile([P, C], F32, tag="bias")
            nc.sync.dma_start(bias_sb[:], bias_b[:, :])
            E_sb = []
            for p in range(NPAIR):
                e = res.tile([H, P], BF, tag=f"E{p}")
                nc.sync.dma_start(e[:], E_const[p])
                E_sb.append(e)

            # zero-init of cc-dependent tiles hoisted here: no dependency,
            # keeps the post-collective DVE critical path to just the copies
            ctxs_bd = res.tile([P, C], BF, tag="ctxs_bd")
            nc.vector.memset(ctxs_bd[:], 0.0)
            Ksel = []
            for kc in range(KC):
                s = res.tile([P, H], BF, tag=f"Ksel{kc}", name=f"Ksel{kc}")
                nc.vector.memset(s[:], 0.0)
                Ksel.append(s)

            # ---- phase 1: kv, elu(k), ctx+ksum ----
            # v tiles are resident with a constant ones column per pair, so
            # each pair's ctx matmul also accumulates ksum (col 128); the
            # diagonal 64x64 blocks hold the two heads' ctx, off-diagonal
            # blocks are ignored garbage.
            VN = 3
            v_res = [res.tile([P, CP], BF, tag=f"vres{r}", name=f"v_res{r}")
                     for r in range(VN)]
            for r in range(VN):
                ones_view = v_res[r][:].rearrange(
                    "p (g c) -> p g c", c=P + 1)[:, :, P : P + 1]
                nc.vector.memset(ones_view, 1.0)


            qte = [[None] * KC for _ in range(NCH)]

            def qt_chunk(j):
                for mc in range(KC):
                    pq = pmm.tile([P, CH], F32, tag="mm")
                    for kc in range(KC):
                        nc.tensor.matmul(pq[:], qw_sb[kc][:, ts(mc, P)],
                                         tgtT_sb[kc][:, ts(j, CH)],
                                         start=(kc == 0), stop=(kc == KC - 1))
                    mn = tmp.tile([P, CH], BF, tag="mn")
                    nc.scalar.activation(mn[:], pq[:],
                                         mybir.ActivationFunctionType.Relu,
                                         scale=-1.0)
                    ex = tmp.tile([P, CH], BF, tag="ex")
                    nc.scalar.activation(ex[:], mn[:],
                                         mybir.ActivationFunctionType.Exp,
                                         scale=-1.0)
                    q_sb = qtep.tile([P, CH], BF, tag=f"qte{j}_{mc}",
                                     name=f"qte{j}_{mc}")
                    nc.vector.scalar_tensor_tensor(
                        q_sb[:], pq[:], 0.0, ex[:],
                        mybir.AluOpType.max, mybir.AluOpType.add)
                    qte[j][mc] = q_sb

            pacc = tc.alloc_tile_pool(name="acc", bufs=1, space="PSUM")
            ctx_ps = [pacc.tile([P, P + 1], F32, tag=f"ctx{p}",
                                name=f"ctx_ps{p}") for p in range(NPAIR)]
            for i in range(NT1):
                pk = pmm.tile([P, C], F32, tag="mm")
                pv = pmm.tile([P, C], F32, tag="mm")
                for kc in range(KC):
                    lhsT = refT_sb[kc][:, ts(i, P)]
                    nc.tensor.matmul(pk[:], lhsT, kvw_sb[kc][:, 0:C],
                                     start=(kc == 0), stop=(kc == KC - 1))
                    nc.tensor.matmul(pv[:], lhsT, kvw_sb[kc][:, C : 2 * C],
                                     start=(kc == 0), stop=(kc == KC - 1))
                # elu(x)+1 = max(x,0) + exp(min(x, 0));  exp(min(x,0)) =
                # exp(-relu(-x)) as two chained ACT ops, one DVE op
                mn = tmp.tile([P, C], BF, tag="mn")
                nc.scalar.activation(mn[:], pk[:],
                                     mybir.ActivationFunctionType.Relu,
                                     scale=-1.0)
                ex = tmp.tile([P, C], BF, tag="ex")
                nc.scalar.activation(ex[:], mn[:],
                                     mybir.ActivationFunctionType.Exp,
                                     scale=-1.0)
                k_sb = kvp.tile([P, C], BF, tag="k")
                nc.vector.scalar_tensor_tensor(
                    k_sb[:], pk[:], 0.0, ex[:],
                    mybir.AluOpType.max, mybir.AluOpType.add)
                v_sb = v_res[i % VN]
                v_view = v_sb[:].rearrange("p (g c) -> p g c",
                                           c=P + 1)[:, :, 0:P]
                nc.scalar.activation(
                    v_view, pv[:].rearrange("p (g c) -> p g c", c=P),
                    mybir.ActivationFunctionType.Copy)
                # ctx+ksum accumulate per head pair (one matmul each)
                for p in range(NPAIR):
                    nc.tensor.matmul(
                        ctx_ps[p][:], k_sb[:, ts(p, P)],
                        v_sb[:, p * (P + 1) : (p + 1) * (P + 1)],
                        start=(i == 0), stop=(i == NT1 - 1))


            # ---- collective: pair AllReduce of ctx + ksum ----
            ctx_cp = res.tile([P, CP], F32, tag="ctx_cp")
            for p in range(NPAIR):
                nc.scalar.activation(ctx_cp[:, ts(p, P + 1)], ctx_ps[p][:],
                                     mybir.ActivationFunctionType.Copy)
            pacc.release()
            nc.sync.dma_start(
                cc_in[:].rearrange("(p f) -> p f", p=P), ctx_cp[:])
            nc.gpsimd.collective_compute(
                "AllReduce", mybir.AluOpType.add,
                replica_groups=replica_groups,
                ins=[cc_in[:]], outs=[cc_out[:]])

            def build_state():
                # collective results -> ctxs_bd (block-diagonal pair blocks,
                # one matmul computes both heads' x) and Ksel columns
                ctxr = res.tile([P, CP], F32, tag="ctxr", name="ctxr")
                nc.sync.dma_start(
                    ctxr[:], cc_out[:].rearrange("(p f) -> p f", p=P))
                for p in range(NPAIR):
                    q0 = p * (P + 1)
                    nc.gpsimd.tensor_copy(ctxs_bd[0:D, p * P : p * P + D],
                                          ctxr[0:D, q0 : q0 + D])
                    nc.gpsimd.tensor_copy(
                        ctxs_bd[D:P, p * P + D : (p + 1) * P],
                        ctxr[D:P, q0 + D : q0 + P])
                for kc in range(KC):
                    kq = kc * (P + 1) + P
                    nc.gpsimd.tensor_copy(Ksel[kc][0:D, 2 * kc : 2 * kc + 1],
                                          ctxr[0:D, kq : kq + 1])
                    nc.gpsimd.tensor_copy(
                        Ksel[kc][D:P, 2 * kc + 1 : 2 * kc + 2],
                        ctxr[D:P, kq : kq + 1])

            # ---- phase 2b: A(j) = denom/recip/x per chunk, B(j) = out-proj;
            # emitted with `lookahead` A-stages ahead of each B-stage so the
            # PE stream always has independent matmuls while DVE/ACT finish
            # the previous chunks.
            paux = tc.alloc_tile_pool(name="aux", bufs=1, space="PSUM")

            def stage_a(j):
                den = paux.tile([H, CH], F32, tag="rb", bufs=2, name="den")
                for kc in range(KC):
                    nc.tensor.matmul(den[:], Ksel[kc][:], qte[j][kc][:],
                                     start=(kc == 0), stop=(kc == KC - 1))
                rec = rcp.tile([H, CH], F32, tag="rec")
                nc.vector.tensor_scalar_add(rec[:], den[:], 1e-6)
                rec2 = rcp.tile([H, CH], F32, tag="rec2")
                nc.vector.reciprocal(rec2[:], rec[:])
                recb = rcp.tile([H, CH], BF, tag="recb")
                nc.vector.tensor_scalar_mul(recb[:], rec2[:], SCALE)
                xts = []
                pxs = []
                for p in range(NPAIR):
                    px = paux.tile([P, CH], F32, tag="px", bufs=3,
                                   name="px")
                    nc.tensor.matmul(px[:], ctxs_bd[:, ts(p, P)],
                                     qte[j][p][:], start=True, stop=True)
                    pxs.append(px)
                for p in range(NPAIR):
                    prb = paux.tile([P, CH], F32, tag="rb", bufs=2,
                                    name="prb")
                    nc.tensor.matmul(prb[:], E_sb[p][:], recb[:],
                                     start=True, stop=True)
                    rb = rcp.tile([P, CH], BF, tag="rbs")
                    nc.scalar.activation(rb[:], prb[:],
                                         mybir.ActivationFunctionType.Copy)
                    xt = xtp.tile([P, CH], BF, tag=f"xt{p}")
                    nc.vector.tensor_mul(xt[:], pxs[p][:], rb[:])
                    xts.append(xt)
                return xts

            def stage_b(j, xts):
                for rt in range(RT):
                    po = pmm.tile([P, C], F32, tag="mm", name="po")
                    for kc in range(KC):
                        nc.tensor.matmul(po[:], xts[kc][:, ts(rt, P)],
                                         pw_sb[kc][:], start=(kc == 0),
                                         stop=(kc == KC - 1))
                    o_sb = op_.tile([P, C], F32, tag="o")
                    nc.vector.tensor_add(o_sb[:], po[:], bias_sb[:])
                    nc.sync.dma_start(out_ext[ts(j * RT + rt, P), :], o_sb[:])

            for j in range(NCH):
                if j == max(NCH - 2, 0):
                    build_state()
                qt_chunk(j)
            pend = []
            for j in range(NCH):
                pend.append((j, stage_a(j)))
                if len(pend) > lookahead:
                    jj, xx = pend.pop(0)
                    stage_b(jj, xx)
            for jj, xx in pend:
                stage_b(jj, xx)
            paux.release()
    nc.compile()
    return nc


def _shard_inputs(target_data, reference_data, q_w, kv_w, proj_w, proj_b,
                  R, ncores):
    bf = ml_dtypes.bfloat16
    kv_wT = np.ascontiguousarray(kv_w.T).astype(bf)
    q_wT = np.ascontiguousarray(q_w.T).astype(bf)
    proj_wT = np.ascontiguousarray(proj_w.T).astype(bf)
    bias_b = np.ascontiguousarray(
        np.broadcast_to(np.asarray(proj_b)[None, :], (128, C))).astype(
            np.float32)
    npair = H // 2
    E_const = np.zeros((npair, H, 128), dtype=bf)
    for p in range(npair):
        E_const[p, 2 * p, 0:D] = 1.0
        E_const[p, 2 * p + 1, D:128] = 1.0
    in_maps = []
    for c in range(ncores):
        b, half = divmod(c, 2)
        sl = slice(half * R, (half + 1) * R)
        in_maps.append({
            "refT": np.ascontiguousarray(
                np.asarray(reference_data)[b, sl, :].T).astype(bf),
            "tgtT": np.ascontiguousarray(
                np.asarray(target_data)[b, sl, :].T).astype(bf),
            "kv_wT": kv_wT, "q_wT": q_wT, "proj_wT": proj_wT,
            "bias_b": bias_b, "E_const": E_const,
        })
    return in_maps


def kernel(target_data, reference_data, q_w, kv_w, proj_w, proj_b):
    R = M // 2
    key = (R, NCORES)
    if key not in _CACHE:
        _CACHE[key] = build(R, R, NCORES,
                            [[0, 1], [2, 3], [4, 5], [6, 7]], lookahead=3)
    nc = _CACHE[key]
    in_maps = _shard_inputs(target_data, reference_data, q_w, kv_w, proj_w,
                            proj_b, R, NCORES)
    res = run_bass_kernel_spmd(nc, in_maps, list(range(NCORES)))
    out = np.empty((B, M, C), dtype=np.float32)
    for c in range(NCORES):
        b, half = divmod(c, 2)
        out[b, half * R : (half + 1) * R, :] = res.results[c]["out"]
    return out



# revision 15
# speedup vs baseline: 1.1289x; 1.1289x over previous
"""Trainium2 Bass kernel for linear attention (elu+1 feature map).

Reference computation (B=4, N=M=8192, C=512, H=8, D=64):
    kv   = ref @ kv_w.T              -> k, v  [B,H,N,D]
    q    = tgt @ q_w.T               -> [B,H,M,D];  q,k -> elu(x)+1
    ctx  = sum_n k v^T per head      -> [B,H,D,D];  ksum = sum_n k
    x    = (q @ ctx) * SCALE / (1e-6 + q . ksum)
    out  = x @ proj_w.T + proj_b     -> [B,M,C]

Sharding: 8 cores = 4 batches x 2 row-halves. Each core computes partial
ctx/ksum from its half of N, pair-AllReduces the tiny per-head state, then
produces its half of M rows of the output.

Fast path vs the bf16 baseline:
  - kv / q projections run as fp8(e4m3) DoubleRow matmuls: weights host-
    scaled x32 into e4m3, activations cast straight to e4m3; the 1/32
    rides on the ACT scale of the elu ops (k, q) or Wp' copy-out (v path).
  - elu(x)+1 = min(max(x+1, 1), exp(x)): ex=Exp on ACT; the x+1 stage
    alternates between ACT (Copy scale+bias) and DVE (2-scalar ts) to
    balance engines; combine is one bf16 stt on DVE.
  - Out-projection weight folded with the attention state once per core:
    Wp' = ctx^T @ proj_w rows (per pair); phase 2 multiplies qs = qte *
    bcast(SCALE/den) straight into the out-proj (bf16), no x tensor.
  - ones-column of v carries 8.0 (=1/SCALE) so po comes out true-scale;
    proj_b is accumulated into po by a row-tiled bf16 bias matmul, making
    the final evacuation a plain PSUM->SBUF copy (split ACT/DVE).
  - 1024-wide PSUM tiles (2 banks) halve per-op engine overheads.
"""

import numpy as np
import ml_dtypes

import concourse.bass as bass
import concourse.mybir as mybir
from concourse import bacc
from concourse.tile import TileContext
from concourse.bass import ts
from concourse.bass_utils import run_bass_kernel_spmd

B, N, M, C, H = 4, 8192, 8192, 512, 8
D = C // H
SCALE = D**-0.5
NCORES = 8
BF = mybir.dt.bfloat16
F32 = mybir.dt.float32
F8 = mybir.dt.float8e4

WS = 32.0           # host scale on kv_w / q_w for e4m3
C_ONES = 1.0 / SCALE  # ones-column value: rec = SCALE/den -> po true scale

_CACHE = {}


def build(R_ref, R_q, num_devices, replica_groups, debug=False):
    """Emit the SPMD graph. R_ref/R_q = rows of the ref/target shard."""
    P = 128
    KC = C // P          # 4 c-chunks
    NQ = KC // 2         # 2 chunk-pairs (DoubleRow)
    NT1 = R_ref // (2 * P)  # phase-1 double-row-tile groups (16)
    CH = 512             # phase-2 chunk (columns of positions)
    NCH = R_q // CH      # phase-2 chunks (8)
    RT = CH // P         # row tiles per chunk (4)
    NPAIR = H // 2       # head pairs (4)
    CP = C + NPAIR       # 516: 4 pairs x 129 cols (128 ctx + 1 ksum)
    STATE = P * CP
    DR = mybir.MatmulPerfMode.DoubleRow
    AF = mybir.ActivationFunctionType
    AL = mybir.AluOpType

    nc = bacc.Bacc("TRN2", target_bir_lowering=False, debug=False,
                   num_devices=num_devices)

    refT8 = nc.dram_tensor("refT8", [C, R_ref], F8, kind="ExternalInput")
    tgtT8 = nc.dram_tensor("tgtT8", [C, R_q], F8, kind="ExternalInput")
    kvwT8 = nc.dram_tensor("kvwT8", [C, 2 * C], F8, kind="ExternalInput")
    qwT8 = nc.dram_tensor("qwT8", [C, C], F8, kind="ExternalInput")
    pwT = nc.dram_tensor("pwT", [C, C], BF, kind="ExternalInput")
    bias_b = nc.dram_tensor("bias_b", [P, C], BF, kind="ExternalInput")
    ident = nc.dram_tensor("ident", [P, P], BF, kind="ExternalInput")
    E_in = nc.dram_tensor("E_in", [P, P], BF, kind="ExternalInput")
    out_ext = nc.dram_tensor("out", [R_q, C], F32, kind="ExternalOutput")
    cc_in = nc.dram_tensor("cc_in", [STATE], F32)
    cc_out = nc.dram_tensor("cc_out", [STATE], F32)
    if debug:
        dbg_ctx = nc.dram_tensor("dbg_ctx", [P, CP], F32, kind="ExternalOutput")
        dbg_qte = nc.dram_tensor("dbg_qte", [P, 2 * CH], F32, kind="ExternalOutput")
        dbg_den = nc.dram_tensor("dbg_den", [P, 2 * CH], F32, kind="ExternalOutput")
        dbg_recb = nc.dram_tensor("dbg_recb", [P, 2 * CH], F32, kind="ExternalOutput")
        dbg_xs = nc.dram_tensor("dbg_xs", [P, CH], F32, kind="ExternalOutput")
        dbg_wp = nc.dram_tensor("dbg_wp", [P, C], F32, kind="ExternalOutput")
        dbg_k = nc.dram_tensor("dbg_k", [P, 2 * C], F32, kind="ExternalOutput")

    with TileContext(nc) as tc:
        with (
            tc.tile_pool(name="res", bufs=1) as res,
            tc.tile_pool(name="mm", bufs=2, space="PSUM") as pmm,
            tc.tile_pool(name="kv", bufs=3) as kvp,
            tc.tile_pool(name="tmp", bufs=4) as tmp,
            tc.tile_pool(name="rc", bufs=2) as rcp,
            tc.tile_pool(name="qte", bufs=1) as qtep,
            tc.tile_pool(name="xs", bufs=1) as xsp,
            tc.tile_pool(name="o", bufs=4) as op_,
        ):
            # ---- resident inputs (fp8 chunk-pair layout [128, 2, *]) ----
            NPIECE = 8
            PC_R = R_ref // NPIECE
            PC_Q = R_q // NPIECE
            kvw_int = [res.tile([P, 2, 2 * C], F8, tag=f"kvw{q}",
                                name=f"kvw_int{q}") for q in range(NQ)]
            for q in range(NQ):
                for t in range(2):
                    nc.sync.dma_start(kvw_int[q][:, t, 0:C],
                                      kvwT8[ts(2 * q + t, P), 0:C])
            ref_int = [res.tile([P, 2, R_ref], F8, tag=f"ref{q}",
                                name=f"ref_int{q}") for q in range(NQ)]
            for q in range(NQ):
                for t in range(2):
                    nc.sync.dma_start(ref_int[q][:, t, ts(0, PC_R)],
                                      refT8[ts(2 * q + t, P), ts(0, PC_R)])
            for q in range(NQ):
                for t in range(2):
                    nc.sync.dma_start(kvw_int[q][:, t, C : 2 * C],
                                      kvwT8[ts(2 * q + t, P), C : 2 * C])
            for pc in range(1, NPIECE):
                for q in range(NQ):
                    for t in range(2):
                        nc.sync.dma_start(
                            ref_int[q][:, t, ts(pc, PC_R)],
                            refT8[ts(2 * q + t, P), ts(pc, PC_R)])
            qw_int = [res.tile([P, 2, C], F8, tag=f"qw{q}", name=f"qw_int{q}")
                      for q in range(NQ)]
            for q in range(NQ):
                for t in range(2):
                    nc.sync.dma_start(qw_int[q][:, t, :],
                                      qwT8[ts(2 * q + t, P), :])
            pw_sb = []
            for p in range(NPAIR):
                t_ = res.tile([P, C], BF, tag=f"pw{p}")
                nc.sync.dma_start(t_[:], pwT[ts(p, P), :])
                pw_sb.append(t_)
            tgt_int = [res.tile([P, 2, R_q], F8, tag=f"tgt{q}",
                                name=f"tgt_int{q}") for q in range(NQ)]
            for pc in range(NPIECE):
                for q in range(NQ):
                    for t in range(2):
                        nc.sync.dma_start(
                            tgt_int[q][:, t, ts(pc, PC_Q)],
                            tgtT8[ts(2 * q + t, P), ts(pc, PC_Q)])
            bias_sb = res.tile([P, C], BF, tag="bias")
            nc.sync.dma_start(bias_sb[:], bias_b[:, :])
            ident_sb = res.tile([P, P], BF, tag="ident")
            nc.sync.dma_start(ident_sb[:], ident[:, :])
            E_all = res.tile([P, P], BF, tag="E_all")
            nc.sync.dma_start(E_all[:], E_in[:, :])

            # Ksel32[kc]: [128, 32] cols 0,1 = ksum of heads 2kc, 2kc+1
            Ksel32 = []
            for kc in range(KC):
                s = res.tile([P, 32], BF, tag=f"Ksel{kc}", name=f"Ksel{kc}")
                nc.vector.memset(s[:], 0.0)
                Ksel32.append(s)

            # ---- phase 1: kv (fp8 DR), elu(k), ctx+ksum ----
            # processed in groups of TWO row tiles; pk2/pv2 span 2 PSUM banks
            VN = 3
            v_res = [res.tile([P, 2, CP], BF, tag=f"vres{r}",
                              name=f"v_res{r}") for r in range(VN)]
            for r in range(VN):
                for t in range(2):
                    ones_view = v_res[r][:, t, :].rearrange(
                        "p (g c) -> p g c", c=P + 1)[:, :, P : P + 1]
                    nc.vector.memset(ones_view, C_ONES)

            # ctx accumulators: 2 pairs packed per PSUM bank
            pacc = tc.alloc_tile_pool(name="acc", bufs=1, space="PSUM")
            ctx_ps2 = [pacc.tile([P, 2, P + 1], F32, tag=f"ctx{g}",
                                 name=f"ctx_ps{g}") for g in range(2)]

            def elu(ps, wide, kq, use_act_c1):
                """elu(x)+1 from psum (holds 32x) -> kq bf16 [P, wide]."""
                ex = tmp.tile([P, wide], BF, tag="ex")
                nc.scalar.activation(ex[:], ps[:], AF.Exp, scale=1.0 / WS)
                c1 = tmp.tile([P, wide], BF, tag="c1")
                if use_act_c1:
                    nc.scalar.activation(c1[:], ps[:], AF.Copy,
                                         scale=1.0 / WS, bias=1.0)
                else:
                    nc.vector.tensor_scalar(
                        c1[:], ps[:], 1.0 / WS, 1.0, AL.mult, AL.add)
                nc.vector.scalar_tensor_tensor(
                    kq[:], c1[:], 1.0, ex[:], AL.max, AL.min)
                return kq

            for i in range(NT1):
                pk2 = pmm.tile([P, 2 * C], F32, tag="mmk")
                pv2 = pmm.tile([P, 2 * C], F32, tag="mmv", bufs=1)
                for h in range(2):
                    for q in range(NQ):
                        lhsT = ref_int[q][:, :, ts(2 * i + h, P)]
                        nc.tensor.matmul(pk2[:, ts(h, C)], lhsT,
                                         kvw_int[q][:, :, 0:C],
                                         start=(q == 0), stop=(q == NQ - 1),
                                         perf_mode=DR)
                        nc.tensor.matmul(pv2[:, ts(h, C)], lhsT,
                                         kvw_int[q][:, :, C : 2 * C],
                                         start=(q == 0), stop=(q == NQ - 1),
                                         perf_mode=DR)
                k_sb = elu(pk2, 2 * C, kvp.tile([P, 2 * C], BF, tag="k", name="k_sb"),
                           use_act_c1=(i % 2 == 0))
                if debug and i == 0:
                    dk = res.tile([P, 2 * C], F32, tag="dk")
                    nc.vector.tensor_copy(dk[:], k_sb[:])
                    nc.sync.dma_start(dbg_k[:, :], dk[:])
                v_sb = v_res[i % VN]
                v_view = v_sb[:].rearrange("p t (g c) -> p t g c",
                                           c=P + 1)[:, :, :, 0:P]
                nc.vector.tensor_copy(
                    v_view, pv2[:].rearrange("p (t g c) -> p t g c",
                                             t=2, c=P))
                for h in range(2):
                    for p in range(NPAIR):
                        nc.tensor.matmul(
                            ctx_ps2[p // 2][:, p % 2, :],
                            k_sb[:, h * C + p * P : h * C + (p + 1) * P],
                            v_sb[:, h, p * (P + 1) : (p + 1) * (P + 1)],
                            start=(i == 0 and h == 0),
                            stop=(i == NT1 - 1 and h == 1))

            # ---- collective: pair AllReduce of ctx + ksum (x32 v-scale) ----
            ctx_cp = res.tile([P, CP], F32, tag="ctx_cp")
            for g in range(2):
                nc.scalar.activation(ctx_cp[:, ts(g, 2 * (P + 1))],
                                     ctx_ps2[g][:], AF.Copy)
            pacc.release()
            if debug:
                dcp = res.tile([P, CP], F32, tag="dcp")
                nc.vector.tensor_copy(dcp[:], ctx_cp[:])
                nc.sync.dma_start(dbg_ctx[:, :], dcp[:])
            nc.sync.dma_start(
                cc_in[:].rearrange("(p f) -> p f", p=P), ctx_cp[:])
            nc.gpsimd.collective_compute(
                "AllReduce", AL.add,
                replica_groups=replica_groups,
                ins=[cc_in[:]], outs=[cc_out[:]])

            # ---- phase 2a: qT fp8 DR + elu -> qte (overlaps collective) ----
            # two chunks at a time: pq2 [P, 1024] spans 2 banks
            qte2 = [[None] * KC for _ in range(NCH // 2)]

            def qt_chunk2(j2):
                for mc in range(KC):
                    pq2 = pmm.tile([P, 2 * CH], F32, tag="mmk")
                    for h in range(2):
                        for q in range(NQ):
                            nc.tensor.matmul(
                                pq2[:, ts(h, CH)], qw_int[q][:, :, ts(mc, P)],
                                tgt_int[q][:, :, ts(2 * j2 + h, CH)],
                                start=(q == 0), stop=(q == NQ - 1),
                                perf_mode=DR)
                    q_sb = qtep.tile([P, 2 * CH], BF, tag=f"qte{j2}_{mc}",
                                     name=f"qte{j2}_{mc}")
                    elu(pq2, 2 * CH, q_sb, use_act_c1=(mc % 2 == 0))
                    qte2[j2][mc] = q_sb
                    if debug and j2 == 0 and mc == 0:
                        dq = res.tile([P, 2 * CH], F32, tag="dq")
                        nc.vector.tensor_copy(dq[:], q_sb[:])
                        nc.sync.dma_start(dbg_qte[:, :], dq[:])

            # ---- post-collective state: Ksel, Wp' = ctx^T @ pw (folded) ----
            Wp_bf = [res.tile([P, C], BF, tag=f"Wp{p}", name=f"Wp{p}")
                     for p in range(NPAIR)]

            def build_state():
                ctxr = res.tile([P, CP], F32, tag="ctxr", name="ctxr")
                nc.sync.dma_start(
                    ctxr[:], cc_out[:].rearrange("(p f) -> p f", p=P))
                for kc in range(KC):
                    kq = kc * (P + 1) + P
                    nc.gpsimd.tensor_copy(Ksel32[kc][0:D, 0:1],
                                          ctxr[0:D, kq : kq + 1])
                    nc.gpsimd.tensor_copy(Ksel32[kc][D:P, 1:2],
                                          ctxr[D:P, kq : kq + 1])
                paux2 = tc.alloc_tile_pool(name="aux2", bufs=1, space="PSUM")
                for p in range(NPAIR):
                    # block-diagonal only: off-diagonal 64x64 blocks of the
                    # pair ctx are cross-head garbage and must not enter Wp'
                    ctxb = tmp.tile([P, P], BF, tag="ctxb")
                    nc.vector.memset(ctxb[:], 0.0)
                    q0 = p * (P + 1)
                    nc.scalar.activation(ctxb[0:D, 0:D],
                                         ctxr[0:D, q0 : q0 + D], AF.Copy)
                    nc.scalar.activation(ctxb[D:P, D:P],
                                         ctxr[D:P, q0 + D : q0 + P], AF.Copy)
                    ctxT_ps = paux2.tile([P, P], BF, tag="ctxT", bufs=1,
                                         name="ctxT")
                    nc.tensor.transpose(ctxT_ps[:], ctxb[:], ident_sb[:])
                    ctxT_sb = tmp.tile([P, P], BF, tag="ctxTs")
                    nc.scalar.activation(ctxT_sb[:], ctxT_ps[:], AF.Copy)
                    wpp = paux2.tile([P, C], F32, tag="wpp", bufs=1,
                                     name="wpp")
                    nc.tensor.matmul(wpp[:], ctxT_sb[:], pw_sb[p][:],
                                     start=True, stop=True)
                    # 1/WS folds out the v-path x32
                    nc.scalar.activation(Wp_bf[p][:], wpp[:], AF.Copy,
                                         scale=1.0 / WS)
                    if debug and p == 0:
                        dw = res.tile([P, C], F32, tag="dw")
                        nc.scalar.activation(dw[:], wpp[:], AF.Copy,
                                             scale=1.0 / WS)
                        nc.sync.dma_start(dbg_wp[:, :], dw[:])
                paux2.release()

            for j2 in range(NCH // 2):
                if j2 == max(NCH // 2 - 1, 0):
                    build_state()
                qt_chunk2(j2)

            # ---- phase 2b ----
            # PSUM: po2 reuses tag mmk (4 banks), den2 reuses tag mmv
            # (2 banks), prb pool 2 banks -> 8 total.
            pprb = tc.alloc_tile_pool(name="prb", bufs=2, space="PSUM")

            def stage_a2(j2):
                den2 = pmm.tile([P, 2 * CH], F32, tag="mmv", bufs=1,
                                name="den2")
                for h in range(2):
                    for kc in range(KC):
                        nc.tensor.matmul(
                            den2[ts(kc, 32), ts(h, CH)], Ksel32[kc][:],
                            qte2[j2][kc][:, ts(h, CH)],
                            start=True, stop=True,
                            tile_position=(0, 32 * kc))
                if debug and j2 == 0:
                    dd = res.tile([P, 2 * CH], F32, tag="dd")
                    nc.vector.tensor_copy(dd[:], den2[:])
                    nc.sync.dma_start(dbg_den[:, :], dd[:])
                rec = rcp.tile([P, 2 * CH], F32, tag="rec")
                nc.vector.reciprocal_approx_fast(out=rec[:], in_=den2[:])
                recb = rcp.tile([P, 2 * CH], BF, tag="recb")
                nc.vector.tensor_copy(recb[:], rec[:])
                if debug and j2 == 0:
                    dr = res.tile([P, 2 * CH], F32, tag="dr")
                    nc.vector.tensor_copy(dr[:], recb[:])
                    nc.sync.dma_start(dbg_recb[:, :], dr[:])
                xs2 = []
                for h in range(2):
                    for p in range(NPAIR):
                        prb = pprb.tile([P, CH], F32, tag="prb",
                                        name="prb")
                        nc.tensor.matmul(
                            prb[:], E_all[32 * p : 32 * p + 2, :],
                            recb[32 * p : 32 * p + 2, ts(h, CH)],
                            start=True, stop=True,
                            tile_position=(32 * p, 0))
                        # qs = qte * (SCALE/den)  (bf16)
                        x_sb = xsp.tile([P, CH], BF, tag=f"xs{j2}_{h}_{p}",
                                        name=f"xs{j2}_{h}_{p}")
                        nc.vector.scalar_tensor_tensor(
                            x_sb[:], prb[:], 1.0,
                            qte2[j2][p][:, ts(h, CH)], AL.mult, AL.mult)
                        if debug and j2 == 0 and h == 0 and p == 0:
                            dx = res.tile([P, CH], F32, tag="dx")
                            nc.vector.tensor_copy(dx[:], x_sb[:])
                            nc.sync.dma_start(dbg_xs[:, :], dx[:])
                        xs2.append(x_sb)
                return xs2

            def stage_b2(j2, xs2):
                # 8 row tiles across the two chunks; po2 packs two row tiles
                for g in range(RT):
                    po2 = pmm.tile([P, 2 * C], F32, tag="mmk", name="po2")
                    for rr in range(2):
                        idx = 2 * g + rr     # row tile within chunk pair
                        h = idx // RT        # which chunk of the pair
                        rt = idx % RT
                        out_sl = po2[:, ts(rr, C)]
                        # bias via ones-row matmul (row-tiled concurrent)
                        nc.tensor.matmul(
                            out_sl, E_ones[32 * rt : 32 * rt + 1, :],
                            bias_sb[32 * rt : 32 * rt + 1, :],
                            start=True, stop=False,
                            tile_position=(32 * rt, 0))
                        for p in range(NPAIR):
                            nc.tensor.matmul(
                                out_sl, xs2[4 * h + p][:, ts(rt, P)],
                                Wp_bf[p][:], start=False,
                                stop=(p == NPAIR - 1))
                    o_sb = op_.tile([P, 2 * C], F32, tag="o")
                    if g % 2 == 0:
                        nc.scalar.activation(o_sb[:], po2[:], AF.Copy)
                    else:
                        nc.vector.tensor_copy(o_sb[:], po2[:])
                    for rr in range(2):
                        idx = 2 * g + rr
                        h = idx // RT
                        rt = idx % RT
                        nc.sync.dma_start(
                            out_ext[ts((2 * j2 + h) * RT + rt, P), :],
                            o_sb[:, ts(rr, C)])

            # all-ones rows for the bias matmul
            E_ones = res.tile([P, P], BF, tag="E_ones")
            nc.vector.memset(E_ones[:], 1.0)

            pend = []
            for j2 in range(NCH // 2):
                pend.append((j2, stage_a2(j2)))
            for j2, xs2 in pend:
                stage_b2(j2, xs2)
            pprb.release()
    nc.compile()
    return nc


def _shard_inputs(target_data, reference_data, q_w, kv_w, proj_w, proj_b,
                  R, ncores):
    bf = ml_dtypes.bfloat16
    f8 = ml_dtypes.float8_e4m3fn
    kvwT8 = np.ascontiguousarray(kv_w.T * WS).astype(f8)
    qwT8 = np.ascontiguousarray(q_w.T * WS).astype(f8)
    pwT = np.ascontiguousarray(proj_w.T).astype(bf)
    bias_b = np.ascontiguousarray(
        np.broadcast_to(np.asarray(proj_b)[None, :], (128, C))).astype(bf)
    ident = np.eye(128, dtype=np.float32).astype(bf)
    E_np = np.zeros((128, 128), dtype=np.float32)
    for p in range(4):
        for t in range(2):
            E_np[32 * p + t, 64 * t : 64 * t + 64] = 1.0
    E_np = E_np.astype(bf)
    in_maps = []
    for c in range(ncores):
        b, half = divmod(c, 2)
        sl = slice(half * R, (half + 1) * R)
        in_maps.append({
            "refT8": np.ascontiguousarray(
                np.asarray(reference_data)[b, sl, :].T).astype(f8),
            "tgtT8": np.ascontiguousarray(
                np.asarray(target_data)[b, sl, :].T).astype(f8),
            "kvwT8": kvwT8, "qwT8": qwT8, "pwT": pwT,
            "bias_b": bias_b, "ident": ident, "E_in": E_np,
        })
    return in_maps


def kernel(target_data, reference_data, q_w, kv_w, proj_w, proj_b):
    R = M // 2
    key = (R, NCORES)
    if key not in _CACHE:
        _CACHE[key] = build(R, R, NCORES,
                            [[0, 1], [2, 3], [4, 5], [6, 7]])
    nc = _CACHE[key]
    in_maps = _shard_inputs(target_data, reference_data, q_w, kv_w, proj_w,
                            proj_b, R, NCORES)
    res = run_bass_kernel_spmd(nc, in_maps, list(range(NCORES)))
    out = np.empty((B, M, C), dtype=np.float32)
    for c in range(NCORES):
        b, half = divmod(c, 2)
        out[b, half * R : (half + 1) * R, :] = res.results[c]["out"]
    return out
